# revision 10
# baseline (speedup 1.0000x reference)
"""Trainium2 Bass kernel for nn_ConformalLayers (8-core data-parallel).

Math (reference):
    X = x.reshape(B, 3072).T                         # [3072, B]
    Y = M @ X                                        # [16384, B]
    Y_extra = s * ||X||_col + sum((T @ X) * X, 0)    # [1, B]
    out = (Y / Y_extra).T.reshape(B, 64, 16, 16)

Sharding: batch B=4096 split as 512 columns per core; M^T / T^T / s
replicated. Each core computes out^T rows [512, 16384] locally; host
concatenates. All GEMMs run on the PE array in float32r (full-rate fp32
with tf32-like accumulate, ~1e-4 rel err). Per core:
    GEMM1: out[b, m] = sum_k X[k, b] * MT[k, m]   (lhsT = X tile, rhs = MT)
    GEMM2: Z^T[b, k'] = sum_k X[k, b] * TT[k, k'] -> q[b] = sum_k' Z^T * x_nat
    norm:  n2[b] = sum_k x_nat[b, k]^2  (ACT Square with accум)
    R = 1 / (s * sqrt(n2) + q);  out rows scaled by R during PSUM eviction.
"""

import os
from contextlib import ExitStack

import numpy as np

import concourse.bass as bass
import concourse.tile as tile
from concourse import bacc, mybir
from concourse import bass_utils
from concourse.kernels.tile_matmul import (
    batched_consumer,
    batched_producer_kxn,
    batched_reducer,
    composable_matmul_tile_kernel,
    dma_from_dram_kxm,
    dma_from_dram_kxn,
    dma_to_dram_mxn,
)

B = 4096
IN_NUMEL = 3072
OUT_NUMEL = 16384
OUT_DIMS = (64, 16, 16)
N_CORES = 8
BC = B // N_CORES            # 512 batch columns per core
P = 128
NB = BC // P                 # 4 batch blocks of 128
NT_T = IN_NUMEL // 512       # 6 n-tiles over T columns
_MM_DT_NAME = os.environ.get("KERNEL_MM_DT", "float32r")
MM_DT = getattr(mybir.dt, _MM_DT_NAME)   # matmul dtype
_MM_NP_DT = mybir.dt.np(MM_DT)

_PROGRAM = None
# Dev bisect knob: full | gemm1 | gemm1_scaled | noTTR | noRecip
_VARIANT = os.environ.get("KERNEL_VARIANT", "full")


def _build_program():
    nc = bacc.Bacc(
        "TRN2",
        target_bir_lowering=False,
        debug=False,
        enable_asserts=False,
        num_devices=N_CORES,
        enable_partition_id=False,
    )
    xt = nc.dram_tensor("xt", (IN_NUMEL, BC), MM_DT, kind="ExternalInput")
    xn = nc.dram_tensor("xn", (BC, IN_NUMEL), mybir.dt.float32, kind="ExternalInput")
    mt = nc.dram_tensor("mt", (IN_NUMEL, OUT_NUMEL), MM_DT, kind="ExternalInput")
    tt = nc.dram_tensor("tt", (IN_NUMEL, IN_NUMEL), MM_DT, kind="ExternalInput")
    sc = nc.dram_tensor("sc", (P, 1), mybir.dt.float32, kind="ExternalInput")
    out = nc.dram_tensor("out", (BC, OUT_NUMEL), mybir.dt.float32, kind="ExternalOutput")

    f32 = mybir.dt.float32
    Alu = mybir.AluOpType
    Act = mybir.ActivationFunctionType

    with tile.TileContext(nc) as tc:
        with ExitStack() as ctx:
            small = ctx.enter_context(tc.tile_pool(name="small", bufs=1))
            xn_pool = ctx.enter_context(tc.tile_pool(name="xnp", bufs=1))
            scratch = ctx.enter_context(tc.tile_pool(name="scr", bufs=2))
            kxm_pool = ctx.enter_context(tc.tile_pool(name="kxm", bufs=7))
            kxn_pool = ctx.enter_context(tc.tile_pool(name="kxn", bufs=8))

            # x natural layout, [p, b_block, k]; row b_block*128+p of x slice.
            xn_t = xn_pool.tile([P, NB, IN_NUMEL], f32)
            xn_ap = xn.ap().rearrange("(t p) k -> p t k", p=P)
            for c in range(NT_T):
                nc.sync.dma_start(
                    xn_t[:, :, c * 512 : (c + 1) * 512],
                    xn_ap[:, :, c * 512 : (c + 1) * 512],
                )
            s_sb = small.tile([P, 1], f32)
            nc.sync.dma_start(s_sb[:], sc.ap())

            np2 = small.tile([P, NB * NT_T], f32)   # per-chunk sum(x^2)
            qp = small.tile([P, NB * NT_T], f32)    # per-chunk sum(Z*x)
            n2 = small.tile([P, NB], f32)
            qv = small.tile([P, NB], f32)
            sn = small.tile([P, NB], f32)
            ye = small.tile([P, NB], f32)
            rt = small.tile([P, NB], f32)           # 1 / Y_extra

            # column norms^2 of X == row norms^2 of x_nat (free-dim reduce)
            if _VARIANT not in ("gemm1", "gemm1_scaled"):
                for b in range(NB):
                    for c in range(NT_T):
                        scr = scratch.tile([P, 512], f32, tag="sq")
                        nc.scalar.activation(
                            scr[:],
                            xn_t[:, b, c * 512 : (c + 1) * 512],
                            Act.Square,
                            accum_out=np2[:, b * NT_T + c : b * NT_T + c + 1],
                        )
            if _VARIANT == "gemm1_scaled":
                nc.vector.memset(rt[:], 1.0)

            kxm_producer, kxm_shape = dma_from_dram_kxm(kxm_pool, xt.ap())
            tt_producer, tt_shape = dma_from_dram_kxn(kxn_pool, tt.ap())
            mt_producer, mt_shape = dma_from_dram_kxn(kxn_pool, mt.ap())
            kxn_producer, kxn_shape = batched_producer_kxn(
                [tt_producer, mt_producer], [tt_shape, mt_shape], batch_dim="n"
            )

            r_emitted = [False]

            def emit_r():
                if r_emitted[0]:
                    return
                r_emitted[0] = True
                for b in range(NB):
                    nc.vector.tensor_reduce(
                        n2[:, b : b + 1], np2[:, b * NT_T : (b + 1) * NT_T],
                        mybir.AxisListType.X, Alu.add,
                    )
                    nc.vector.tensor_reduce(
                        qv[:, b : b + 1], qp[:, b * NT_T : (b + 1) * NT_T],
                        mybir.AxisListType.X, Alu.add,
                    )
                nc.scalar.sqrt(sn[:], n2[:])
                # ye = sn * s + q
                nc.vector.scalar_tensor_tensor(
                    out=ye[:], in0=sn[:], scalar=s_sb[:, 0:1], in1=qv[:],
                    op0=Alu.mult, op1=Alu.add,
                )
                if _VARIANT == "noRecip":
                    nc.vector.tensor_copy(rt[:], ye[:])
                else:
                    nc.vector.reciprocal(rt[:], ye[:])

            def reducer_tt(nc_, psum, sbuf_slice, md):
                idx = md.m_subtile_idx * NT_T + md.n_tile_idx
                if _VARIANT == "noTTR":
                    nc_.vector.tensor_copy(sbuf_slice, psum)
                    nc_.vector.tensor_reduce(
                        qp[:, idx : idx + 1], psum,
                        mybir.AxisListType.X, Alu.add,
                    )
                    return
                nc_.vector.tensor_mul(
                    sbuf_slice,
                    psum,
                    xn_t[:, md.m_subtile_idx,
                         md.n_tile_idx * 512 : (md.n_tile_idx + 1) * 512],
                )
                nc_.vector.tensor_reduce(
                    qp[:, idx : idx + 1], sbuf_slice,
                    mybir.AxisListType.X, Alu.add,
                )

            def reducer_mt(nc_, psum, sbuf_slice, md):
                if _VARIANT in ("gemm1",):
                    nc_.scalar.copy(sbuf_slice, psum)
                    return
                if _VARIANT not in ("gemm1_scaled",):
                    emit_r()
                nc_.vector.tensor_scalar_mul(
                    sbuf_slice, psum, rt[:, md.m_subtile_idx : md.m_subtile_idx + 1]
                )

            def consumer_noop(nc_, sbuf, md):
                pass

            if _VARIANT in ("gemm1", "gemm1_scaled"):
                kxn_producer, kxn_shape = mt_producer, mt_shape
                reducers = batched_reducer([reducer_mt], "n")
                consumers = batched_consumer([dma_to_dram_mxn(out.ap())], "n")
            else:
                reducers = batched_reducer([reducer_tt, reducer_mt], "n")
                consumers = batched_consumer(
                    [consumer_noop, dma_to_dram_mxn(out.ap())], "n"
                )

            composable_matmul_tile_kernel(
                tc=tc,
                kxm_shape=kxm_shape,
                kxn_shape=kxn_shape,
                output_type=f32,
                kxm_producer=kxm_producer,
                kxn_producer=kxn_producer,
                mxn_subtile_reducer=reducers,
                mxn_consumer=consumers,
                psum_n_bufs=2,
                cache_tiles=True,
            )

    nc.compile()
    return nc


def get_program():
    global _PROGRAM
    if _PROGRAM is None:
        _PROGRAM = _build_program()
    return _PROGRAM


def make_in_maps(x, cached_matrix, cached_matrix_extra, cached_tensor_extra):
    xf = np.ascontiguousarray(np.asarray(x, dtype=np.float32).reshape(B, IN_NUMEL))
    XT = np.ascontiguousarray(xf.T).astype(_MM_NP_DT)
    MT = np.ascontiguousarray(np.asarray(cached_matrix, dtype=np.float32).T).astype(_MM_NP_DT)
    TT = np.ascontiguousarray(np.asarray(cached_tensor_extra, dtype=np.float32).T).astype(_MM_NP_DT)
    s = np.full((P, 1), np.float32(np.asarray(cached_matrix_extra).reshape(-1)[0]),
                dtype=np.float32)
    in_maps = []
    for c in range(N_CORES):
        sl = slice(c * BC, (c + 1) * BC)
        in_maps.append({
            "xt": np.ascontiguousarray(XT[:, sl]),
            "xn": np.ascontiguousarray(xf[sl, :]),
            "mt": MT,
            "tt": TT,
            "sc": s,
        })
    return in_maps


_AXON_EXEC = None


def _build_axon_exec():
    """Staged PJRT runner for the axon path.

    run_bass_kernel_spmd's axon redirect concatenates all per-core inputs into
    single giant host arrays (1.6 GB for the replicated cached_matrix), which
    hits a pathologically slow transfer path in the relay. Instead we stage
    shards/replicas with individually-sized device_puts and run the same
    bass_exec custom call through shard_map ourselves.
    """
    import jax
    from jax.sharding import Mesh, NamedSharding, PartitionSpec
    from jax.experimental.shard_map import shard_map
    from concourse import bass2jax

    nc = get_program()
    bass2jax.install_neuronx_cc_hook()

    in_names, out_names, out_avals = [], [], []
    for alloc in nc.m.functions[0].allocations:
        if not isinstance(alloc, mybir.MemoryLocationSet):
            continue
        name = alloc.memorylocations[0].name
        if alloc.kind == "ExternalInput":
            in_names.append(name)
        elif alloc.kind == "ExternalOutput":
            out_names.append(name)
            out_avals.append(
                jax.core.ShapedArray(
                    tuple(alloc.tensor_shape), mybir.dt.np(alloc.dtype)
                )
            )
    all_in_names = in_names + out_names
    # per-input sharding: batch-sharded vs replicated model caches
    sharded_inputs = {"xt", "xn"}

    def _body(*args):
        outs = bass2jax._bass_exec_p.bind(
            *args,
            out_avals=tuple(out_avals),
            in_names=tuple(all_in_names),
            out_names=tuple(out_names),
            lowering_input_output_aliases=(),
            sim_require_finite=True,
            sim_require_nnan=True,
            nc=nc,
        )
        return tuple(outs)

    devices = jax.devices()[:N_CORES]
    mesh = Mesh(np.asarray(devices), ("core",))
    core_spec = PartitionSpec("core")
    repl_spec = PartitionSpec()
    in_specs = tuple(
        core_spec if n in sharded_inputs else repl_spec for n in in_names
    ) + (core_spec,) * len(out_names)
    sharded = jax.jit(
        shard_map(
            _body,
            mesh=mesh,
            in_specs=in_specs,
            out_specs=(core_spec,) * len(out_names),
            check_rep=False,
        ),
        keep_unused=True,
    )

    def stage(in_maps):
        import concurrent.futures as cf

        core_sh = NamedSharding(mesh, core_spec)
        repl_sh = NamedSharding(mesh, repl_spec)

        def stage_one(name):
            if name in sharded_inputs:
                glob = np.concatenate([m[name] for m in in_maps], axis=0)
                return jax.device_put(glob, core_sh)
            return jax.device_put(in_maps[0][name], repl_sh)

        with cf.ThreadPoolExecutor(len(in_names)) as ex:
            staged = list(ex.map(stage_one, in_names))
        for s in staged:
            s.block_until_ready()
        zeros = [
            jax.jit(
                lambda a=a: jax.numpy.zeros((N_CORES * a.shape[0], *a.shape[1:]), a.dtype),
                out_shardings=core_sh,
            )()
            for a in out_avals
        ]
        return staged + zeros

    def execute(staged):
        outs = sharded(*staged)
        jax.block_until_ready(outs)
        return outs

    def run(in_maps):
        return execute(stage(in_maps))

    _state = {"sharded": sharded, "stage": stage, "execute": execute, "run": run}
    return _state


def get_axon_exec():
    global _AXON_EXEC
    if _AXON_EXEC is None:
        _AXON_EXEC = _build_axon_exec()
    return _AXON_EXEC


def kernel(x, cached_matrix, cached_matrix_extra, cached_tensor_extra):
    from concourse._compat import axon_active

    in_maps = make_in_maps(x, cached_matrix, cached_matrix_extra, cached_tensor_extra)
    if axon_active():
        outs = get_axon_exec()["run"](in_maps)
        out = np.asarray(outs[0])  # [B, OUT_NUMEL]
    else:
        nc = get_program()
        res = bass_utils.run_bass_kernel_spmd(nc, in_maps, core_ids=list(range(N_CORES)))
        out = np.concatenate([r["out"] for r in res.results], axis=0)
    return np.ascontiguousarray(out).reshape(B, *OUT_DIMS)


# revision 11
# speedup vs baseline: 98.3863x; 98.3863x over previous
"""Trainium2 Bass kernel for nn_ConformalLayers (8-core data-parallel).

Math (reference):
    X = x.reshape(B, 3072).T                         # [3072, B]
    Y = M @ X                                        # [16384, B]
    Y_extra = s * ||X||_col + sum((T @ X) * X, 0)    # [1, B]
    out = (Y / Y_extra).T.reshape(B, 64, 16, 16)

Sharding: batch B=4096 split as 512 columns per core; M^T / T^T / s
replicated. Each core computes out^T rows [512, 16384] locally; host
concatenates. All GEMMs run on the PE array in float32r (full-rate fp32
with tf32-like accumulate, ~1e-4 rel err). Per core:
    GEMM1: out[b, m] = sum_k X[k, b] * MT[k, m]   (lhsT = X tile, rhs = MT)
    GEMM2: Z^T[b, k'] = sum_k X[k, b] * TT[k, k'] -> q[b] = sum_k' Z^T * x_nat
    norm:  n2[b] = sum_k x_nat[b, k]^2  (ACT Square with accум)
    R = 1 / (s * sqrt(n2) + q);  out rows scaled by R during PSUM eviction.
"""

import os
from contextlib import ExitStack

import numpy as np

import concourse.bass as bass
import concourse.tile as tile
from concourse import bacc, mybir
from concourse import bass_utils
from concourse.kernels.tile_matmul import (
    batched_consumer,
    batched_producer_kxn,
    batched_reducer,
    composable_matmul_tile_kernel,
    dma_from_dram_kxm,
    dma_from_dram_kxn,
    dma_to_dram_mxn,
)

B = 4096
IN_NUMEL = 3072
OUT_NUMEL = 16384
OUT_DIMS = (64, 16, 16)
N_CORES = 8
BC = B // N_CORES            # 512 batch columns per core
P = 128
NB = BC // P                 # 4 batch blocks of 128
NT_T = IN_NUMEL // 512       # 6 n-tiles over T columns
# float16: same 1 cycle/row PE rate as float32r but with FWL weight loads
# (fp32-class dtypes can't) and half the HBM/transfer bytes on the dominant
# cached_matrix stream; measured 2.9e-4 rel err vs 1.5e-4 for float32r.
_MM_DT_NAME = os.environ.get("KERNEL_MM_DT", "float16")
MM_DT = getattr(mybir.dt, _MM_DT_NAME)   # matmul dtype
_MM_NP_DT = mybir.dt.np(MM_DT)

_PROGRAM = None
# Dev bisect knob: full | gemm1 | gemm1_scaled | noTTR | noRecip
_VARIANT = os.environ.get("KERNEL_VARIANT", "full")


def _build_program():
    nc = bacc.Bacc(
        "TRN2",
        target_bir_lowering=False,
        debug=False,
        enable_asserts=False,
        num_devices=N_CORES,
        enable_partition_id=False,
    )
    xt = nc.dram_tensor("xt", (IN_NUMEL, BC), MM_DT, kind="ExternalInput")
    xn = nc.dram_tensor("xn", (BC, IN_NUMEL), mybir.dt.float32, kind="ExternalInput")
    mt = nc.dram_tensor("mt", (IN_NUMEL, OUT_NUMEL), MM_DT, kind="ExternalInput")
    tt = nc.dram_tensor("tt", (IN_NUMEL, IN_NUMEL), MM_DT, kind="ExternalInput")
    sc = nc.dram_tensor("sc", (P, 1), mybir.dt.float32, kind="ExternalInput")
    out = nc.dram_tensor("out", (BC, OUT_NUMEL), mybir.dt.float32, kind="ExternalOutput")

    f32 = mybir.dt.float32
    Alu = mybir.AluOpType
    Act = mybir.ActivationFunctionType

    with tile.TileContext(nc) as tc:
        with ExitStack() as ctx:
            small = ctx.enter_context(tc.tile_pool(name="small", bufs=1))
            xn_pool = ctx.enter_context(tc.tile_pool(name="xnp", bufs=1))
            scratch = ctx.enter_context(tc.tile_pool(name="scr", bufs=2))
            kxm_pool = ctx.enter_context(tc.tile_pool(name="kxm", bufs=7))
            kxn_pool = ctx.enter_context(tc.tile_pool(name="kxn", bufs=8))

            # x natural layout, [p, b_block, k]; row b_block*128+p of x slice.
            xn_t = xn_pool.tile([P, NB, IN_NUMEL], f32)
            xn_ap = xn.ap().rearrange("(t p) k -> p t k", p=P)
            for c in range(NT_T):
                nc.sync.dma_start(
                    xn_t[:, :, c * 512 : (c + 1) * 512],
                    xn_ap[:, :, c * 512 : (c + 1) * 512],
                )
            s_sb = small.tile([P, 1], f32)
            nc.sync.dma_start(s_sb[:], sc.ap())

            np2 = small.tile([P, NB * NT_T], f32)   # per-chunk sum(x^2)
            qp = small.tile([P, NB * NT_T], f32)    # per-chunk sum(Z*x)
            n2 = small.tile([P, NB], f32)
            qv = small.tile([P, NB], f32)
            sn = small.tile([P, NB], f32)
            ye = small.tile([P, NB], f32)
            rt = small.tile([P, NB], f32)           # 1 / Y_extra

            # column norms^2 of X == row norms^2 of x_nat (free-dim reduce)
            if _VARIANT not in ("gemm1", "gemm1_scaled"):
                for b in range(NB):
                    for c in range(NT_T):
                        scr = scratch.tile([P, 512], f32, tag="sq")
                        nc.scalar.activation(
                            scr[:],
                            xn_t[:, b, c * 512 : (c + 1) * 512],
                            Act.Square,
                            accum_out=np2[:, b * NT_T + c : b * NT_T + c + 1],
                        )
            if _VARIANT == "gemm1_scaled":
                nc.vector.memset(rt[:], 1.0)

            kxm_producer, kxm_shape = dma_from_dram_kxm(kxm_pool, xt.ap())
            tt_producer, tt_shape = dma_from_dram_kxn(kxn_pool, tt.ap())
            mt_producer, mt_shape = dma_from_dram_kxn(kxn_pool, mt.ap())
            kxn_producer, kxn_shape = batched_producer_kxn(
                [tt_producer, mt_producer], [tt_shape, mt_shape], batch_dim="n"
            )

            r_emitted = [False]

            def emit_r():
                if r_emitted[0]:
                    return
                r_emitted[0] = True
                for b in range(NB):
                    nc.vector.tensor_reduce(
                        n2[:, b : b + 1], np2[:, b * NT_T : (b + 1) * NT_T],
                        mybir.AxisListType.X, Alu.add,
                    )
                    nc.vector.tensor_reduce(
                        qv[:, b : b + 1], qp[:, b * NT_T : (b + 1) * NT_T],
                        mybir.AxisListType.X, Alu.add,
                    )
                nc.scalar.sqrt(sn[:], n2[:])
                # ye = sn * s + q
                nc.vector.scalar_tensor_tensor(
                    out=ye[:], in0=sn[:], scalar=s_sb[:, 0:1], in1=qv[:],
                    op0=Alu.mult, op1=Alu.add,
                )
                if _VARIANT == "noRecip":
                    nc.vector.tensor_copy(rt[:], ye[:])
                else:
                    nc.vector.reciprocal(rt[:], ye[:])

            def reducer_tt(nc_, psum, sbuf_slice, md):
                idx = md.m_subtile_idx * NT_T + md.n_tile_idx
                if _VARIANT == "noTTR":
                    nc_.vector.tensor_copy(sbuf_slice, psum)
                    nc_.vector.tensor_reduce(
                        qp[:, idx : idx + 1], psum,
                        mybir.AxisListType.X, Alu.add,
                    )
                    return
                nc_.vector.tensor_mul(
                    sbuf_slice,
                    psum,
                    xn_t[:, md.m_subtile_idx,
                         md.n_tile_idx * 512 : (md.n_tile_idx + 1) * 512],
                )
                nc_.vector.tensor_reduce(
                    qp[:, idx : idx + 1], sbuf_slice,
                    mybir.AxisListType.X, Alu.add,
                )

            def reducer_mt(nc_, psum, sbuf_slice, md):
                if _VARIANT in ("gemm1",):
                    nc_.scalar.copy(sbuf_slice, psum)
                    return
                if _VARIANT not in ("gemm1_scaled",):
                    emit_r()
                nc_.vector.tensor_scalar_mul(
                    sbuf_slice, psum, rt[:, md.m_subtile_idx : md.m_subtile_idx + 1]
                )

            def consumer_noop(nc_, sbuf, md):
                pass

            if _VARIANT in ("gemm1", "gemm1_scaled"):
                kxn_producer, kxn_shape = mt_producer, mt_shape
                reducers = batched_reducer([reducer_mt], "n")
                consumers = batched_consumer([dma_to_dram_mxn(out.ap())], "n")
            else:
                reducers = batched_reducer([reducer_tt, reducer_mt], "n")
                consumers = batched_consumer(
                    [consumer_noop, dma_to_dram_mxn(out.ap())], "n"
                )

            composable_matmul_tile_kernel(
                tc=tc,
                kxm_shape=kxm_shape,
                kxn_shape=kxn_shape,
                output_type=f32,
                kxm_producer=kxm_producer,
                kxn_producer=kxn_producer,
                mxn_subtile_reducer=reducers,
                mxn_consumer=consumers,
                psum_n_bufs=2,
                cache_tiles=True,
            )

    nc.compile()
    return nc


def get_program():
    global _PROGRAM
    if _PROGRAM is None:
        _PROGRAM = _build_program()
    return _PROGRAM


def make_in_maps(x, cached_matrix, cached_matrix_extra, cached_tensor_extra):
    xf = np.ascontiguousarray(np.asarray(x, dtype=np.float32).reshape(B, IN_NUMEL))
    XT = np.ascontiguousarray(xf.T).astype(_MM_NP_DT)
    MT = np.ascontiguousarray(np.asarray(cached_matrix, dtype=np.float32).T).astype(_MM_NP_DT)
    TT = np.ascontiguousarray(np.asarray(cached_tensor_extra, dtype=np.float32).T).astype(_MM_NP_DT)
    s = np.full((P, 1), np.float32(np.asarray(cached_matrix_extra).reshape(-1)[0]),
                dtype=np.float32)
    in_maps = []
    for c in range(N_CORES):
        sl = slice(c * BC, (c + 1) * BC)
        in_maps.append({
            "xt": np.ascontiguousarray(XT[:, sl]),
            "xn": np.ascontiguousarray(xf[sl, :]),
            "mt": MT,
            "tt": TT,
            "sc": s,
        })
    return in_maps


_AXON_EXEC = None


def _build_axon_exec():
    """Staged PJRT runner for the axon path.

    run_bass_kernel_spmd's axon redirect concatenates all per-core inputs into
    single giant host arrays (1.6 GB for the replicated cached_matrix), which
    hits a pathologically slow transfer path in the relay. Instead we stage
    shards/replicas with individually-sized device_puts and run the same
    bass_exec custom call through shard_map ourselves.
    """
    import jax
    from jax.sharding import Mesh, NamedSharding, PartitionSpec
    from jax.experimental.shard_map import shard_map
    from concourse import bass2jax

    nc = get_program()
    bass2jax.install_neuronx_cc_hook()

    in_names, out_names, out_avals = [], [], []
    for alloc in nc.m.functions[0].allocations:
        if not isinstance(alloc, mybir.MemoryLocationSet):
            continue
        name = alloc.memorylocations[0].name
        if alloc.kind == "ExternalInput":
            in_names.append(name)
        elif alloc.kind == "ExternalOutput":
            out_names.append(name)
            out_avals.append(
                jax.core.ShapedArray(
                    tuple(alloc.tensor_shape), mybir.dt.np(alloc.dtype)
                )
            )
    all_in_names = in_names + out_names
    # per-input sharding: batch-sharded vs replicated model caches
    sharded_inputs = {"xt", "xn"}

    def _body(*args):
        outs = bass2jax._bass_exec_p.bind(
            *args,
            out_avals=tuple(out_avals),
            in_names=tuple(all_in_names),
            out_names=tuple(out_names),
            lowering_input_output_aliases=(),
            sim_require_finite=True,
            sim_require_nnan=True,
            nc=nc,
        )
        return tuple(outs)

    devices = jax.devices()[:N_CORES]
    mesh = Mesh(np.asarray(devices), ("core",))
    core_spec = PartitionSpec("core")
    repl_spec = PartitionSpec()
    in_specs = tuple(
        core_spec if n in sharded_inputs else repl_spec for n in in_names
    ) + (core_spec,) * len(out_names)
    sharded = jax.jit(
        shard_map(
            _body,
            mesh=mesh,
            in_specs=in_specs,
            out_specs=(core_spec,) * len(out_names),
            check_rep=False,
        ),
        keep_unused=True,
    )

    def stage(in_maps):
        import concurrent.futures as cf

        core_sh = NamedSharding(mesh, core_spec)
        repl_sh = NamedSharding(mesh, repl_spec)

        def stage_one(name):
            if name in sharded_inputs:
                glob = np.concatenate([m[name] for m in in_maps], axis=0)
                return jax.device_put(glob, core_sh)
            return jax.device_put(in_maps[0][name], repl_sh)

        with cf.ThreadPoolExecutor(len(in_names)) as ex:
            staged = list(ex.map(stage_one, in_names))
        for s in staged:
            s.block_until_ready()
        zeros = [
            jax.jit(
                lambda a=a: jax.numpy.zeros((N_CORES * a.shape[0], *a.shape[1:]), a.dtype),
                out_shardings=core_sh,
            )()
            for a in out_avals
        ]
        return staged + zeros

    def execute(staged):
        outs = sharded(*staged)
        jax.block_until_ready(outs)
        return outs

    def run(in_maps):
        return execute(stage(in_maps))

    _state = {"sharded": sharded, "stage": stage, "execute": execute, "run": run}
    return _state


def get_axon_exec():
    global _AXON_EXEC
    if _AXON_EXEC is None:
        _AXON_EXEC = _build_axon_exec()
    return _AXON_EXEC


def kernel(x, cached_matrix, cached_matrix_extra, cached_tensor_extra):
    from concourse._compat import axon_active

    in_maps = make_in_maps(x, cached_matrix, cached_matrix_extra, cached_tensor_extra)
    if axon_active():
        outs = get_axon_exec()["run"](in_maps)
        out = np.asarray(outs[0])  # [B, OUT_NUMEL]
    else:
        nc = get_program()
        res = bass_utils.run_bass_kernel_spmd(nc, in_maps, core_ids=list(range(N_CORES)))
        out = np.concatenate([r["out"] for r in res.results], axis=0)
    return np.ascontiguousarray(out).reshape(B, *OUT_DIMS)
